# revision 21
# baseline (speedup 1.0000x reference)
"""ActivateAttention Trainium2 kernel — 8 NeuronCores, SPMD, head-sharded.

Sharding: core i handles batch b=i//4 and head-group g=i%4 (3 of the 12
heads: ha=3g, hb=3g+1, hc=3g+2), all 4096 queries, full K/V for its
batch. Each core returns a PARTIAL output x_g @ Wp[:, 192g:192g+192].T;
the host sums the 4 group partials per batch and adds bp.

Per-core pipeline (fp16 compute, f32 PSUM accumulate):
  lead-in: weights + all of V and K projected before attention (x^T via
           PE transposes, W^T.T @ x^T; k gets +bias then exact GELU on
           ACT, grouped so the exp table loads once for attention).
           Input DMAs split across the sync and gpsimd hardware queues;
           transposes rotate through a 3-buf PSUM pool that closes
           before attention and hands its banks to the PV accumulators.
  attn:    8 passes over q-halves (512 cols), merged stream of pair and
           solo steps. Per step the two heads' S^T [128,512] land in
           adjacent PSUM banks from matmuls issued back-to-back into
           DIFFERENT PE row groups (partitions 0-63 / 64-127) so they
           stream concurrently. exp(SCALE*S): ACT exact into pt_a, DVE
           one-instruction Schraudolph exp2 into pt_b (affine f32 ->
           int16 convert, bitcast as fp16) — separate tiles/banks so the
           two engines run in parallel. PV matmuls are emitted one step
           LATE so the in-order PE queue never blocks on the exp
           semaphores. PV accumulates [65,512] per head (ones column ->
           softmax denominators). The solo head hc is row-paired across
           adjacent k-tiles via duplicated qT/kT copies in both halves.
  tail:    per finished q-half: x * recip(denominator) -> xT fp16;
           out_partial = xT.T @ Wp_slice^T (host adds bp).
"""

import numpy as np
from contextlib import ExitStack

from concourse import bass, bacc, mybir, masks, tile
from concourse import bass_utils

F32 = mybir.dt.float32
FP16 = mybir.dt.float16
I16 = mybir.dt.int16
AF = mybir.ActivationFunctionType
ALU = mybir.AluOpType

B = 2
N = 4096
DIM = 768
H = 12
D = 64
SCALE = D ** -0.5            # 1/8
N_CORES = 8
HG = 3                       # heads per core
GD = HG * D                  # 192 output dims per core

NT_K = N // 128              # 32 key row-tiles
NCT = DIM // 128             # 6 input-channel tiles
NQH = N // 512               # 8 query halves
LOG2E = 1.4426950408889634
SCH_A = SCALE * LOG2E * 1024.0
SCH_B = 15.0 * 1024.0 - 46.0


def build_nc() -> bass.Bass:
    nc = bacc.Bacc("TRN2", target_bir_lowering=False, debug=False)

    query = nc.declare_dram_parameter("query", [N, DIM], F32, False).ap()
    key = nc.declare_dram_parameter("key", [N, DIM], F32, False).ap()
    value = nc.declare_dram_parameter("value", [N, DIM], F32, False).ap()
    Wq = nc.declare_dram_parameter("Wq", [GD, DIM], F32, False).ap()
    Wk = nc.declare_dram_parameter("Wk", [GD, DIM], F32, False).ap()
    bk = nc.declare_dram_parameter("bk", [GD], F32, False).ap()
    Wv = nc.declare_dram_parameter("Wv", [GD, DIM], F32, False).ap()
    Wp = nc.declare_dram_parameter("Wp", [DIM, GD], F32, False).ap()
    out = nc.declare_dram_parameter("out", [N, DIM], FP16, True).ap()

    with tile.TileContext(nc) as tc, ExitStack() as ctx:
        # ---------------- persistent SBUF ----------------
        cpool = ctx.enter_context(tc.tile_pool(name="const", bufs=1))
        ident = cpool.tile([128, 128], FP16)
        masks.make_identity(nc, ident[:])
        ones16 = cpool.tile([1, D], FP16)
        nc.vector.memset(ones16[:], 1.0)

        bk_pair = cpool.tile([128, 1], F32)
        nc.sync.dma_start(out=bk_pair[:], in_=bk[0:128].rearrange("(p a) -> p a", a=1))
        bk_solo = cpool.tile([128, 1], F32)
        nc.sync.dma_start(out=bk_solo[0:64, :], in_=bk[128:192].rearrange("(p a) -> p a", a=1))
        nc.sync.dma_start(out=bk_solo[64:128, :], in_=bk[128:192].rearrange("(p a) -> p a", a=1))

        # wqk_t blocks: [0:128) Wq^T pair, [128:256) Wq^T solo dup,
        #               [256:384) Wk^T pair, [384:512) Wk^T solo dup
        wqk_t = cpool.tile([128, NCT, 512], FP16)
        wv_t = cpool.tile([128, NCT, GD], FP16)
        wpA = cpool.tile([128, DIM], FP16)
        wpB = cpool.tile([64, DIM], FP16)

        qT_P = [cpool.tile([128, 1024], FP16, name=f"qTP{j}", tag=f"qTP{j}")
                for j in range(4)]
        qT_S = [cpool.tile([128, 1024], FP16, name=f"qTS{j}", tag=f"qTS{j}")
                for j in range(4)]
        kT_P = [cpool.tile([128, 1024], FP16, name=f"kTP{j}", tag=f"kTP{j}")
                for j in range(4)]
        kT_S = [cpool.tile([128, 1024], FP16, name=f"kTS{j}", tag=f"kTS{j}")
                for j in range(4)]
        v_aug = [cpool.tile([128, HG * 65], FP16, name=f"va{t}", tag=f"va{t}")
                 for t in range(NT_K)]
        xTa = [cpool.tile([128, 512], FP16, name=f"xTa{q}", tag=f"xTa{q}")
               for q in range(NQH)]
        xTb = [cpool.tile([64, 512], FP16, name=f"xTb{q}", tag=f"xTb{q}")
               for q in range(NQH)]

        # ---------------- pools ----------------
        # PSUM banks: spool 2x[128,1024]f32 = 4, rpool 1 canonical = 1,
        # ltp 3 canonical = 3 (lead-in; closed -> apool 3x[65,512]f32 = 3)
        spool = ctx.enter_context(tc.tile_pool(name="spool", bufs=5, space="PSUM"))
        ltp_cm = tc.tile_pool(name="ltp", bufs=3, space="PSUM")
        ltp = ltp_cm.__enter__()
        ldpool = ctx.enter_context(tc.tile_pool(name="ldpool", bufs=6))
        cast_pool = ctx.enter_context(tc.tile_pool(name="cast", bufs=5))
        xt_pool = ctx.enter_context(tc.tile_pool(name="xt", bufs=3))
        pt_pool = ctx.enter_context(tc.tile_pool(name="pt", bufs=9))
        dpool = ctx.enter_context(tc.tile_pool(name="drain", bufs=1))
        opool = ctx.enter_context(tc.tile_pool(name="out", bufs=2))

        # canonical PSUM scratch tile: [128, 512] f32 (exactly one bank),
        # viewed as [128, 6, 128] fp16 for transposes or sliced for misc.
        def canon(pool, name):
            return pool.tile([128, 512], F32, tag="cn", name=name)

        def as_tp(t):
            return t[:].bitcast(FP16)[:, 0:768].rearrange(
                "p (c x) -> p c x", x=128)

        # ---------------- weight prep ----------------
        def load_cast(src_ap, rows, cols, dma_eng):
            wf = ldpool.tile([rows, cols], F32, tag="wf")
            dma_eng.dma_start(out=wf[:], in_=src_ap)
            wb = cast_pool.tile([rows, cols], FP16, tag="wb")
            nc.vector.tensor_copy(wb[:], wf[:])
            return wb

        def transp_to(wb, rows, dst_slices):
            for c in range(NCT):
                tpc = canon(ltp, "tpw")
                tp = tpc[:].bitcast(FP16)[:, 0:rows]
                nc.tensor.transpose(tp, wb[:, 128 * c:128 * (c + 1)],
                                    ident[:rows, :rows])
                for dst in dst_slices:
                    nc.vector.tensor_copy(dst(c), tp)

        for wsrc, col0 in ((Wq, 0), (Wk, 256)):
            wb = load_cast(wsrc[0:128, :], 128, DIM, nc.sync)
            transp_to(wb, 128, [lambda c, col0=col0: wqk_t[:, c, col0:col0 + 128]])
            wb = load_cast(wsrc[128:192, :], 64, DIM, nc.sync)
            transp_to(wb, 64, [
                lambda c, col0=col0: wqk_t[:, c, col0 + 128:col0 + 192],
                lambda c, col0=col0: wqk_t[:, c, col0 + 192:col0 + 256]])
        wb = load_cast(Wv[0:128, :], 128, DIM, nc.gpsimd)
        transp_to(wb, 128, [lambda c: wv_t[:, c, 0:128]])
        wb = load_cast(Wv[128:192, :], 64, DIM, nc.gpsimd)
        transp_to(wb, 64, [lambda c: wv_t[:, c, 128:192]])
        for r in range(NCT):
            wb = load_cast(Wp[128 * r:128 * (r + 1), :], 128, GD, nc.gpsimd)
            tpc = canon(ltp, "tpp")
            tp = tpc[:].bitcast(FP16)[:, 0:128]
            nc.tensor.transpose(tp, wb[:, 0:128], ident[:])
            nc.vector.tensor_copy(wpA[:, 128 * r:128 * (r + 1)], tp)
            tpc2 = canon(ltp, "tpp2")
            tp2 = tpc2[:].bitcast(FP16)[0:64, 128:256]
            nc.tensor.transpose(tp2, wb[:, 128:192], ident[:])
            nc.vector.tensor_copy(wpB[:, 128 * r:128 * (r + 1)], tp2)

        # ---------------- input row-tile -> x^T ----------------
        def emit_x_rowtile(src_ap, xt, j, t, dma_eng, cp_eng, cast_eng, tp_pool):
            row0 = 1024 * j + 128 * t
            xf = ldpool.tile([128, DIM], F32, tag="xf")
            dma_eng.dma_start(out=xf[:], in_=src_ap[row0:row0 + 128, :])
            xb = cast_pool.tile([128, DIM], FP16, tag="xb")
            cast_eng(xb[:], xf[:])
            tpc = canon(tp_pool, "tpx")
            tp = as_tp(tpc)
            for c in range(NCT):
                nc.tensor.transpose(tp[:, c, :], xb[:, 128 * c:128 * (c + 1)],
                                    ident[:])
            cp_eng(xt[:, :, 128 * t:128 * (t + 1)], tp)

        def emit_proj_block(xt, wcol0, dst, gelu, bias, cp_eng=None):
            for h2 in range(2):
                pp = canon(spool, f"pp{h2}")
                for c in range(NCT):
                    nc.tensor.matmul(
                        pp[:], wqk_t[:, c, wcol0:wcol0 + 128],
                        xt[:, c, 512 * h2:512 * (h2 + 1)],
                        start=(c == 0), stop=(c == NCT - 1))
                dsl = dst[:, 512 * h2:512 * (h2 + 1)]
                if gelu:
                    nc.scalar.activation(dsl, pp[:], AF.Gelu, bias=bias,
                                         scale=1.0)
                else:
                    (cp_eng or nc.vector.tensor_copy)(dsl, pp[:])

        def emit_v_tile(t):
            vf = ldpool.tile([128, DIM], F32, tag="vf")
            nc.gpsimd.dma_start(out=vf[:], in_=value[128 * t:128 * (t + 1), :])
            vb = cast_pool.tile([128, DIM], FP16, tag="vb")
            nc.vector.tensor_copy(vb[:], vf[:])
            vt = cast_pool.tile([128, NCT, 128], FP16, tag="vt")
            tpc = canon(spool, "tpv")
            tpv = as_tp(tpc)
            for c in range(NCT):
                nc.tensor.transpose(tpv[:, c, :], vb[:, 128 * c:128 * (c + 1)],
                                    ident[:])
            nc.vector.tensor_copy(vt[:], tpv)
            pvc = canon(spool, "pvt")
            pv = pvc[:, 0:GD]
            for c in range(NCT):
                nc.tensor.matmul(pv, vt[:, c, :], wv_t[:, c, :],
                                 start=(c == 0), stop=(c == NCT - 1))
            dst3 = v_aug[t][:].rearrange("p (h w) -> p h w", w=65)
            nc.scalar.copy(dst3[:, :, 0:64],
                           pv.rearrange("p (h w) -> p h w", w=64))
            nc.vector.memset(dst3[:, :, 64:65], 1.0)

        # ---------------- attention ----------------
        def emit_qk_exp(qh, st, dve_only=False):
            kind, idx = st
            q0 = 512 * (qh % 2)
            jq = qh // 2
            if kind == "P":
                c0 = 128 * (idx % 8)
                jk = idx // 8
                la = kT_P[jk][0:64, c0:c0 + 128]
                lb = kT_P[jk][64:128, c0:c0 + 128]
                ra = qT_P[jq][0:64, q0:q0 + 512]
                rb = qT_P[jq][64:128, q0:q0 + 512]
            else:
                kta, ktb = 2 * idx, 2 * idx + 1
                la = kT_S[kta // 8][0:64, 128 * (kta % 8):128 * (kta % 8) + 128]
                lb = kT_S[ktb // 8][64:128, 128 * (ktb % 8):128 * (ktb % 8) + 128]
                ra = qT_S[jq][0:64, q0:q0 + 512]
                rb = qT_S[jq][64:128, q0:q0 + 512]
            slot_a = canon(spool, f"sa{kind}{qh}_{idx}")
            slot_b = canon(spool, f"sb{kind}{qh}_{idx}")
            nc.tensor.matmul(slot_a[:], la, ra, start=True, stop=True)
            nc.tensor.matmul(slot_b[:], lb, rb, start=True, stop=True)
            pta = pt_pool.tile([128, 512], FP16, tag="pt", name=f"pa{kind}{qh}_{idx}")
            ptb = pt_pool.tile([128, 512], FP16, tag="pt", name=f"pb{kind}{qh}_{idx}")
            if dve_only:
                nc.vector.tensor_scalar(pta[:].bitcast(I16), slot_a[:],
                                        SCH_A, SCH_B, ALU.mult, ALU.add)
            else:
                nc.scalar.activation(pta[:], slot_a[:], AF.Exp, scale=SCALE)
            nc.vector.tensor_scalar(ptb[:].bitcast(I16), slot_b[:],
                                    SCH_A, SCH_B, ALU.mult, ALU.add)
            return (kind, idx, pta, ptb)

        def emit_pv(state, xps_a, xps_b, xps_c):
            kind, idx, pta, ptb = state
            if kind == "P":
                va = v_aug[idx][:]
                nc.tensor.matmul(xps_a[:], va[:, 0:65], pta[:],
                                 start=(idx == 0), stop=(idx == NT_K - 1),
                                 skip_group_check=True)
                nc.tensor.matmul(xps_b[:], va[:, 65:130], ptb[:],
                                 start=(idx == 0), stop=(idx == NT_K - 1),
                                 skip_group_check=True)
            else:
                kta, ktb = 2 * idx, 2 * idx + 1
                nc.tensor.matmul(xps_c[:], v_aug[kta][:, 130:195], pta[:],
                                 start=(idx == 0), stop=False,
                                 skip_group_check=True)
                nc.tensor.matmul(xps_c[:], v_aug[ktb][:, 130:195], ptb[:],
                                 start=False, stop=(idx == NT_K // 2 - 1),
                                 skip_group_check=True)

        def emit_drain(xps, dst):
            d16 = dpool.tile([1, 512], FP16, tag="d16")
            nc.scalar.copy(d16[:], xps[64:65, :])
            rpc = canon(spool, "rps")
            Rp = rpc[0:D, :]
            nc.tensor.matmul(Rp, ones16[:], d16[:], start=True, stop=True)
            Rs = dpool.tile([D, 512], F32, tag="Rs")
            nc.vector.reciprocal_approx_fast(Rs[:], Rp)
            nc.vector.tensor_tensor(dst, xps[0:64, :], Rs[:], op=ALU.mult)

        _ot_state = {}

        def emit_out_piece(qh, piece):
            tq, hi = piece // 2, piece % 2
            if hi == 0:
                _ot_state["ot"] = opool.tile([128, DIM], FP16, tag="ot",
                                             name=f"ot{qh}_{tq}")
            ot = _ot_state["ot"]
            o0, w = (0, 512) if hi == 0 else (512, 256)
            poc = canon(spool, f"po{qh}_{tq}_{o0}")
            po = poc[:, 0:w]
            nc.tensor.matmul(po, xTa[qh][:, 128 * tq:128 * (tq + 1)],
                             wpA[:, o0:o0 + w], start=True, stop=False)
            nc.tensor.matmul(po, xTb[qh][:, 128 * tq:128 * (tq + 1)],
                             wpB[:, o0:o0 + w], start=False, stop=True)
            if tq % 2 == 0:
                nc.scalar.copy(ot[:, o0:o0 + w], po)
            else:
                nc.vector.tensor_copy(ot[:, o0:o0 + w], po)
            if hi == 1:
                nc.gpsimd.dma_start(
                    out=out[512 * qh + 128 * tq:512 * qh + 128 * (tq + 1), :],
                    in_=ot[:])

        # ---------------- lead-in (pass 0 interleaved, all-DVE exp) -------
        steps = []
        for kt in range(NT_K):
            steps.append(("P", kt))
            if kt % 2 == 1:
                steps.append(("S", kt // 2))
        LAG = 4

        p0_acc = {}
        p0_pend = []

        def p0_accums():
            if not p0_acc:
                for nm in ("a", "b", "c"):
                    cnt = canon(ltp, f"x0{nm}")
                    p0_acc[nm] = cnt[0:65, :]
            return p0_acc["a"], p0_acc["b"], p0_acc["c"]

        kxt = qxt = None
        for t in range(NT_K):
            j, r = t // 8, t % 8
            if r == 0:
                kxt = xt_pool.tile([128, NCT, 1024], FP16, tag="xt",
                                   name=f"kxt{j}")
                qxt = xt_pool.tile([128, NCT, 1024], FP16, tag="xt",
                                   name=f"qxt{j}")
            emit_v_tile(t)
            emit_x_rowtile(key, kxt, j, r, nc.sync, nc.scalar.copy,
                           nc.vector.tensor_copy, spool)
            emit_x_rowtile(query, qxt, j, r, nc.sync, nc.scalar.copy,
                           nc.vector.tensor_copy, spool)
            if r == 7:
                emit_proj_block(kxt, 256, kT_P[j], True, bk_pair[:, 0:1])
                emit_proj_block(kxt, 384, kT_S[j], True, bk_solo[:, 0:1])
                emit_proj_block(qxt, 0, qT_P[j], False, None)
                emit_proj_block(qxt, 128, qT_S[j], False, None)
                for st in steps[12 * j:12 * (j + 1)]:
                    p0_pend.append(emit_qk_exp(0, st, dve_only=True))
                    if len(p0_pend) > LAG:
                        emit_pv(p0_pend.pop(0), *p0_accums())
        for state in p0_pend:
            emit_pv(state, *p0_accums())
        xa0, xb0, xc0 = p0_accums()
        emit_drain(xa0, xTa[0][0:64, :])
        emit_drain(xb0, xTa[0][64:128, :])
        emit_drain(xc0, xTb[0][0:64, :])

        ltp_cm.__exit__(None, None, None)
        apool = ctx.enter_context(tc.tile_pool(name="apool", bufs=3, space="PSUM"))

        pend_drain = None
        for qh in range(1, NQH):
            xps_a = apool.tile([65, 512], F32, tag="ap", name=f"xa{qh}")
            xps_b = apool.tile([65, 512], F32, tag="ap", name=f"xb{qh}")
            xps_c = apool.tile([65, 512], F32, tag="ap", name=f"xc{qh}")
            pend = []
            for si, st in enumerate(steps):
                pend.append(emit_qk_exp(qh, st))
                if pend_drain is not None and si in (3, 5, 7):
                    emit_drain(*pend_drain.pop(0))
                    if not pend_drain:
                        pend_drain = None
                if si >= LAG:
                    emit_pv(pend.pop(0), xps_a, xps_b, xps_c)
                if qh >= 1 and si in (10, 16, 22, 28):
                    base = {10: 0, 16: 2, 22: 4, 28: 6}[si]
                    emit_out_piece(qh - 1, base)
                    emit_out_piece(qh - 1, base + 1)
            for state in pend:
                emit_pv(state, xps_a, xps_b, xps_c)
            pend_drain = [(xps_a[:], xTa[qh][0:64, :]),
                          (xps_b[:], xTa[qh][64:128, :]),
                          (xps_c[:], xTb[qh][0:64, :])]
        for xps, dst in pend_drain:
            emit_drain(xps, dst)
        for piece in range(8):
            emit_out_piece(NQH - 1, piece)

    nc.compile()
    return nc


_NC_CACHE = {}


def _get_nc():
    if "nc" not in _NC_CACHE:
        _NC_CACHE["nc"] = build_nc()
    return _NC_CACHE["nc"]


def make_in_maps(query, key, value, Wq, Wk, bk, Wv, Wp):
    in_maps = []
    for i in range(N_CORES):
        b, g = i // 4, i % 4
        gs = slice(GD * g, GD * (g + 1))
        in_maps.append({
            "query": query[b], "key": key[b], "value": value[b],
            "Wq": Wq[gs, :], "Wk": Wk[gs, :], "bk": bk[gs],
            "Wv": Wv[gs, :], "Wp": Wp[:, gs],
        })
    return [{k: np.ascontiguousarray(v, dtype=np.float32)
             for k, v in m.items()} for m in in_maps]


def reduce_out(res, bp):
    out = np.empty((B, N, DIM), dtype=np.float32)
    for b in range(B):
        acc = res.results[4 * b]["out"].astype(np.float32).copy()
        for g in range(1, 4):
            acc += res.results[4 * b + g]["out"]
        out[b] = acc + bp
    return out


def kernel(query, key, value, Wq, Wk, bk, Wv, Wp, bp, _results_hook=None):
    args = [np.asarray(a, dtype=np.float32)
            for a in (query, key, value, Wq, Wk, bk, Wv, Wp)]
    nc = _get_nc()
    in_maps = make_in_maps(*args)
    res = bass_utils.run_bass_kernel_spmd(nc, in_maps,
                                          core_ids=list(range(N_CORES)))
    if _results_hook is not None:
        _results_hook(res)
    return reduce_out(res, np.asarray(bp, dtype=np.float32))


# revision 22
# speedup vs baseline: 1.0328x; 1.0328x over previous
"""ActivateAttention Trainium2 kernel — 8 NeuronCores, SPMD, head-sharded.

Sharding: core i handles batch b=i//4 and head-group g=i%4 (3 of the 12
heads: ha=3g, hb=3g+1, hc=3g+2), all 4096 queries, full K/V for its
batch. Each core returns a PARTIAL output x_g @ Wp[:, 192g:192g+192].T;
the host sums the 4 group partials per batch and adds bp.

Per-core pipeline (fp16 compute, f32 PSUM accumulate):
  lead-in: weights + all of V and K projected before attention (x^T via
           PE transposes, W^T.T @ x^T; k gets +bias then exact GELU on
           ACT, grouped so the exp table loads once for attention).
           Input DMAs split across the sync and gpsimd hardware queues;
           transposes rotate through a 3-buf PSUM pool that closes
           before attention and hands its banks to the PV accumulators.
  attn:    8 passes over q-halves (512 cols), merged stream of pair and
           solo steps. Per step the two heads' S^T [128,512] land in
           adjacent PSUM banks from matmuls issued back-to-back into
           DIFFERENT PE row groups (partitions 0-63 / 64-127) so they
           stream concurrently. exp(SCALE*S): ACT exact into pt_a, DVE
           one-instruction Schraudolph exp2 into pt_b (affine f32 ->
           int16 convert, bitcast as fp16) — separate tiles/banks so the
           two engines run in parallel. PV matmuls are emitted one step
           LATE so the in-order PE queue never blocks on the exp
           semaphores. PV accumulates [65,512] per head (ones column ->
           softmax denominators). The solo head hc is row-paired across
           adjacent k-tiles via duplicated qT/kT copies in both halves.
  tail:    per finished q-half: x * recip(denominator) -> xT fp16;
           out_partial = xT.T @ Wp_slice^T (host adds bp).
"""

import numpy as np
from contextlib import ExitStack

from concourse import bass, bacc, mybir, masks, tile
from concourse import bass_utils

F32 = mybir.dt.float32
FP16 = mybir.dt.float16
I16 = mybir.dt.int16
AF = mybir.ActivationFunctionType
ALU = mybir.AluOpType

B = 2
N = 4096
DIM = 768
H = 12
D = 64
SCALE = D ** -0.5            # 1/8
N_CORES = 8
HG = 3                       # heads per core
GD = HG * D                  # 192 output dims per core

NT_K = N // 128              # 32 key row-tiles
NCT = DIM // 128             # 6 input-channel tiles
NQH = N // 512               # 8 query halves
LOG2E = 1.4426950408889634
SCH_A = SCALE * LOG2E * 1024.0
SCH_B = 15.0 * 1024.0 - 46.0


def build_nc() -> bass.Bass:
    nc = bacc.Bacc("TRN2", target_bir_lowering=False, debug=False)

    query = nc.declare_dram_parameter("query", [N, DIM], F32, False).ap()
    key = nc.declare_dram_parameter("key", [N, DIM], F32, False).ap()
    value = nc.declare_dram_parameter("value", [N, DIM], F32, False).ap()
    Wq = nc.declare_dram_parameter("Wq", [GD, DIM], F32, False).ap()
    Wk = nc.declare_dram_parameter("Wk", [GD, DIM], F32, False).ap()
    bk = nc.declare_dram_parameter("bk", [GD], F32, False).ap()
    Wv = nc.declare_dram_parameter("Wv", [GD, DIM], F32, False).ap()
    Wp = nc.declare_dram_parameter("Wp", [DIM, GD], F32, False).ap()
    out = nc.declare_dram_parameter("out", [N, DIM], FP16, True).ap()

    with tile.TileContext(nc) as tc, ExitStack() as ctx:
        # ---------------- persistent SBUF ----------------
        cpool = ctx.enter_context(tc.tile_pool(name="const", bufs=1))
        ident = cpool.tile([128, 128], FP16)
        masks.make_identity(nc, ident[:])
        ones16 = cpool.tile([1, D], FP16)
        nc.vector.memset(ones16[:], 1.0)

        bk_pair = cpool.tile([128, 1], F32)
        nc.sync.dma_start(out=bk_pair[:], in_=bk[0:128].rearrange("(p a) -> p a", a=1))
        bk_solo = cpool.tile([128, 1], F32)
        nc.sync.dma_start(out=bk_solo[0:64, :], in_=bk[128:192].rearrange("(p a) -> p a", a=1))
        nc.sync.dma_start(out=bk_solo[64:128, :], in_=bk[128:192].rearrange("(p a) -> p a", a=1))

        # wqk_t blocks: [0:128) Wq^T pair, [128:256) Wq^T solo dup,
        #               [256:384) Wk^T pair, [384:512) Wk^T solo dup
        wqk_t = cpool.tile([128, NCT, 512], FP16)
        wv_t = cpool.tile([128, NCT, GD], FP16)
        wpA = cpool.tile([128, DIM], FP16)
        wpB = cpool.tile([64, DIM], FP16)

        qT_P = [cpool.tile([128, 1024], FP16, name=f"qTP{j}", tag=f"qTP{j}")
                for j in range(4)]
        qT_S = [cpool.tile([128, 1024], FP16, name=f"qTS{j}", tag=f"qTS{j}")
                for j in range(4)]
        kT_P = [cpool.tile([128, 1024], FP16, name=f"kTP{j}", tag=f"kTP{j}")
                for j in range(4)]
        kT_S = [cpool.tile([128, 1024], FP16, name=f"kTS{j}", tag=f"kTS{j}")
                for j in range(4)]
        v_aug = [cpool.tile([128, HG * 65], FP16, name=f"va{t}", tag=f"va{t}")
                 for t in range(NT_K)]
        xTa = [cpool.tile([128, 512], FP16, name=f"xTa{q}", tag=f"xTa{q}")
               for q in range(NQH)]
        xTb = [cpool.tile([64, 512], FP16, name=f"xTb{q}", tag=f"xTb{q}")
               for q in range(NQH)]

        # ---------------- pools ----------------
        # PSUM banks: spool 2x[128,1024]f32 = 4, rpool 1 canonical = 1,
        # ltp 3 canonical = 3 (lead-in; closed -> apool 3x[65,512]f32 = 3)
        spool = ctx.enter_context(tc.tile_pool(name="spool", bufs=5, space="PSUM"))
        ltp_cm = tc.tile_pool(name="ltp", bufs=3, space="PSUM")
        ltp = ltp_cm.__enter__()
        ldpool = ctx.enter_context(tc.tile_pool(name="ldpool", bufs=6))
        cast_pool = ctx.enter_context(tc.tile_pool(name="cast", bufs=5))
        xt_pool = ctx.enter_context(tc.tile_pool(name="xt", bufs=3))
        pt_pool = ctx.enter_context(tc.tile_pool(name="pt", bufs=9))
        dpool = ctx.enter_context(tc.tile_pool(name="drain", bufs=1))
        opool = ctx.enter_context(tc.tile_pool(name="out", bufs=2))

        # canonical PSUM scratch tile: [128, 512] f32 (exactly one bank),
        # viewed as [128, 6, 128] fp16 for transposes or sliced for misc.
        def canon(pool, name):
            return pool.tile([128, 512], F32, tag="cn", name=name)

        def as_tp(t):
            return t[:].bitcast(FP16)[:, 0:768].rearrange(
                "p (c x) -> p c x", x=128)

        # ---------------- weight prep ----------------
        def load_cast(src_ap, rows, cols, dma_eng):
            wf = ldpool.tile([rows, cols], F32, tag="wf")
            dma_eng.dma_start(out=wf[:], in_=src_ap)
            wb = cast_pool.tile([rows, cols], FP16, tag="wb")
            nc.vector.tensor_copy(wb[:], wf[:])
            return wb

        def transp_to(wb, rows, dst_slices):
            for c in range(NCT):
                tpc = canon(ltp, "tpw")
                tp = tpc[:].bitcast(FP16)[:, 0:rows]
                nc.tensor.transpose(tp, wb[:, 128 * c:128 * (c + 1)],
                                    ident[:rows, :rows])
                for dst in dst_slices:
                    nc.vector.tensor_copy(dst(c), tp)

        for wsrc, col0 in ((Wq, 0), (Wk, 256)):
            wb = load_cast(wsrc[0:128, :], 128, DIM, nc.sync)
            transp_to(wb, 128, [lambda c, col0=col0: wqk_t[:, c, col0:col0 + 128]])
            wb = load_cast(wsrc[128:192, :], 64, DIM, nc.sync)
            transp_to(wb, 64, [
                lambda c, col0=col0: wqk_t[:, c, col0 + 128:col0 + 192],
                lambda c, col0=col0: wqk_t[:, c, col0 + 192:col0 + 256]])
        wb = load_cast(Wv[0:128, :], 128, DIM, nc.gpsimd)
        transp_to(wb, 128, [lambda c: wv_t[:, c, 0:128]])
        wb = load_cast(Wv[128:192, :], 64, DIM, nc.gpsimd)
        transp_to(wb, 64, [lambda c: wv_t[:, c, 128:192]])
        for r in range(NCT):
            wb = load_cast(Wp[128 * r:128 * (r + 1), :], 128, GD, nc.gpsimd)
            tpc = canon(ltp, "tpp")
            tp = tpc[:].bitcast(FP16)[:, 0:128]
            nc.tensor.transpose(tp, wb[:, 0:128], ident[:])
            nc.vector.tensor_copy(wpA[:, 128 * r:128 * (r + 1)], tp)
            tpc2 = canon(ltp, "tpp2")
            tp2 = tpc2[:].bitcast(FP16)[0:64, 128:256]
            nc.tensor.transpose(tp2, wb[:, 128:192], ident[:])
            nc.vector.tensor_copy(wpB[:, 128 * r:128 * (r + 1)], tp2)

        # ---------------- input row-tile -> x^T ----------------
        def emit_x_rowtile(src_ap, xt, j, t, dma_eng, cp_eng, cast_eng, tp_pool):
            row0 = 1024 * j + 128 * t
            xf = ldpool.tile([128, DIM], F32, tag="xf")
            dma_eng.dma_start(out=xf[:], in_=src_ap[row0:row0 + 128, :])
            xb = cast_pool.tile([128, DIM], FP16, tag="xb")
            cast_eng(xb[:], xf[:])
            tpc = canon(tp_pool, "tpx")
            tp = as_tp(tpc)
            for c in range(NCT):
                nc.tensor.transpose(tp[:, c, :], xb[:, 128 * c:128 * (c + 1)],
                                    ident[:])
            cp_eng(xt[:, :, 128 * t:128 * (t + 1)], tp)

        def emit_proj_block(xt, wcol0, dst, gelu, bias, cp_eng=None):
            for h2 in range(2):
                pp = canon(spool, f"pp{h2}")
                for c in range(NCT):
                    nc.tensor.matmul(
                        pp[:], wqk_t[:, c, wcol0:wcol0 + 128],
                        xt[:, c, 512 * h2:512 * (h2 + 1)],
                        start=(c == 0), stop=(c == NCT - 1))
                dsl = dst[:, 512 * h2:512 * (h2 + 1)]
                if gelu:
                    nc.scalar.activation(dsl, pp[:], AF.Gelu, bias=bias,
                                         scale=1.0)
                else:
                    (cp_eng or nc.vector.tensor_copy)(dsl, pp[:])

        def emit_v_tile(t):
            vf = ldpool.tile([128, DIM], F32, tag="vf")
            nc.gpsimd.dma_start(out=vf[:], in_=value[128 * t:128 * (t + 1), :])
            vb = cast_pool.tile([128, DIM], FP16, tag="vb")
            nc.vector.tensor_copy(vb[:], vf[:])
            vt = cast_pool.tile([128, NCT, 128], FP16, tag="vt")
            tpc = canon(spool, "tpv")
            tpv = as_tp(tpc)
            for c in range(NCT):
                nc.tensor.transpose(tpv[:, c, :], vb[:, 128 * c:128 * (c + 1)],
                                    ident[:])
            nc.vector.tensor_copy(vt[:], tpv)
            pvc = canon(spool, "pvt")
            pv = pvc[:, 0:GD]
            for c in range(NCT):
                nc.tensor.matmul(pv, vt[:, c, :], wv_t[:, c, :],
                                 start=(c == 0), stop=(c == NCT - 1))
            dst3 = v_aug[t][:].rearrange("p (h w) -> p h w", w=65)
            nc.scalar.copy(dst3[:, :, 0:64],
                           pv.rearrange("p (h w) -> p h w", w=64))
            nc.vector.memset(dst3[:, :, 64:65], 1.0)

        # ---------------- attention ----------------
        def emit_qk_exp(qh, st, dve_only=False):
            kind, idx = st
            q0 = 512 * (qh % 2)
            jq = qh // 2
            if kind == "P":
                c0 = 128 * (idx % 8)
                jk = idx // 8
                la = kT_P[jk][0:64, c0:c0 + 128]
                lb = kT_P[jk][64:128, c0:c0 + 128]
                ra = qT_P[jq][0:64, q0:q0 + 512]
                rb = qT_P[jq][64:128, q0:q0 + 512]
            else:
                kta, ktb = 2 * idx, 2 * idx + 1
                la = kT_S[kta // 8][0:64, 128 * (kta % 8):128 * (kta % 8) + 128]
                lb = kT_S[ktb // 8][64:128, 128 * (ktb % 8):128 * (ktb % 8) + 128]
                ra = qT_S[jq][0:64, q0:q0 + 512]
                rb = qT_S[jq][64:128, q0:q0 + 512]
            slot_a = canon(spool, f"sa{kind}{qh}_{idx}")
            slot_b = canon(spool, f"sb{kind}{qh}_{idx}")
            nc.tensor.matmul(slot_a[:], la, ra, start=True, stop=True)
            nc.tensor.matmul(slot_b[:], lb, rb, start=True, stop=True)
            pta = pt_pool.tile([128, 512], FP16, tag="pt", name=f"pa{kind}{qh}_{idx}")
            ptb = pt_pool.tile([128, 512], FP16, tag="pt", name=f"pb{kind}{qh}_{idx}")
            if dve_only:
                nc.vector.tensor_scalar(pta[:].bitcast(I16), slot_a[:],
                                        SCH_A, SCH_B, ALU.mult, ALU.add)
            else:
                nc.scalar.activation(pta[:], slot_a[:], AF.Exp, scale=SCALE)
            nc.vector.tensor_scalar(ptb[:].bitcast(I16), slot_b[:],
                                    SCH_A, SCH_B, ALU.mult, ALU.add)
            return (kind, idx, pta, ptb)

        def emit_pv(state, xps_a, xps_b, xps_c):
            kind, idx, pta, ptb = state
            if kind == "P":
                va = v_aug[idx][:]
                nc.tensor.matmul(xps_a[:], va[:, 0:65], pta[:],
                                 start=(idx == 0), stop=(idx == NT_K - 1),
                                 skip_group_check=True)
                nc.tensor.matmul(xps_b[:], va[:, 65:130], ptb[:],
                                 start=(idx == 0), stop=(idx == NT_K - 1),
                                 skip_group_check=True)
            else:
                kta, ktb = 2 * idx, 2 * idx + 1
                nc.tensor.matmul(xps_c[:], v_aug[kta][:, 130:195], pta[:],
                                 start=(idx == 0), stop=False,
                                 skip_group_check=True)
                nc.tensor.matmul(xps_c[:], v_aug[ktb][:, 130:195], ptb[:],
                                 start=False, stop=(idx == NT_K // 2 - 1),
                                 skip_group_check=True)

        def emit_drain(xps, dst):
            d16 = dpool.tile([1, 512], FP16, tag="d16")
            nc.scalar.copy(d16[:], xps[64:65, :])
            rpc = canon(spool, "rps")
            Rp = rpc[0:D, :]
            nc.tensor.matmul(Rp, ones16[:], d16[:], start=True, stop=True)
            Rs = dpool.tile([D, 512], F32, tag="Rs")
            nc.vector.reciprocal_approx_fast(Rs[:], Rp)
            nc.vector.tensor_tensor(dst, xps[0:64, :], Rs[:], op=ALU.mult)

        _ot_state = {}

        def emit_out_piece(qh, piece):
            tq, hi = piece // 2, piece % 2
            if hi == 0:
                _ot_state["ot"] = opool.tile([128, DIM], FP16, tag="ot",
                                             name=f"ot{qh}_{tq}")
            ot = _ot_state["ot"]
            o0, w = (0, 512) if hi == 0 else (512, 256)
            poc = canon(spool, f"po{qh}_{tq}_{o0}")
            po = poc[:, 0:w]
            nc.tensor.matmul(po, xTa[qh][:, 128 * tq:128 * (tq + 1)],
                             wpA[:, o0:o0 + w], start=True, stop=False)
            nc.tensor.matmul(po, xTb[qh][:, 128 * tq:128 * (tq + 1)],
                             wpB[:, o0:o0 + w], start=False, stop=True)
            if tq % 2 == 0:
                nc.scalar.copy(ot[:, o0:o0 + w], po)
            else:
                nc.vector.tensor_copy(ot[:, o0:o0 + w], po)
            if hi == 1:
                nc.gpsimd.dma_start(
                    out=out[512 * qh + 128 * tq:512 * qh + 128 * (tq + 1), :],
                    in_=ot[:])

        # ---------------- lead-in (pass 0 interleaved, all-DVE exp) -------
        steps = []
        for kt in range(NT_K):
            steps.append(("P", kt))
            if kt % 2 == 1:
                steps.append(("S", kt // 2))
        LAG = 4

        p0_acc = {}
        p0_pend = []

        def p0_accums():
            if not p0_acc:
                for nm in ("a", "b", "c"):
                    cnt = canon(ltp, f"x0{nm}")
                    p0_acc[nm] = cnt[0:65, :]
            return p0_acc["a"], p0_acc["b"], p0_acc["c"]

        kxt = qxt = None
        p0_ready = []

        def p0_run(n):
            for _ in range(min(n, len(p0_ready))):
                p0_pend.append(emit_qk_exp(0, p0_ready.pop(0), dve_only=True))
                if len(p0_pend) > LAG:
                    emit_pv(p0_pend.pop(0), *p0_accums())

        for t in range(NT_K):
            j, r = t // 8, t % 8
            if r == 0:
                kxt = xt_pool.tile([128, NCT, 1024], FP16, tag="xt",
                                   name=f"kxt{j}")
                qxt = xt_pool.tile([128, NCT, 1024], FP16, tag="xt",
                                   name=f"qxt{j}")
            emit_v_tile(t)
            emit_x_rowtile(key, kxt, j, r, nc.sync, nc.scalar.copy,
                           nc.vector.tensor_copy, spool)
            p0_run(1)
            emit_x_rowtile(query, qxt, j, r, nc.sync, nc.scalar.copy,
                           nc.vector.tensor_copy, spool)
            if r % 2 == 1:
                p0_run(1)
            if r == 7:
                emit_proj_block(kxt, 256, kT_P[j], True, bk_pair[:, 0:1])
                emit_proj_block(kxt, 384, kT_S[j], True, bk_solo[:, 0:1])
                emit_proj_block(qxt, 0, qT_P[j], False, None)
                emit_proj_block(qxt, 128, qT_S[j], False, None)
                p0_ready.extend(steps[12 * j:12 * (j + 1)])
        while p0_ready:
            p0_run(4)
        for state in p0_pend:
            emit_pv(state, *p0_accums())
        xa0, xb0, xc0 = p0_accums()
        emit_drain(xa0, xTa[0][0:64, :])
        emit_drain(xb0, xTa[0][64:128, :])
        emit_drain(xc0, xTb[0][0:64, :])

        ltp_cm.__exit__(None, None, None)
        apool = ctx.enter_context(tc.tile_pool(name="apool", bufs=3, space="PSUM"))

        pend_drain = None
        for qh in range(1, NQH):
            xps_a = apool.tile([65, 512], F32, tag="ap", name=f"xa{qh}")
            xps_b = apool.tile([65, 512], F32, tag="ap", name=f"xb{qh}")
            xps_c = apool.tile([65, 512], F32, tag="ap", name=f"xc{qh}")
            pend = []
            for si, st in enumerate(steps):
                pend.append(emit_qk_exp(qh, st))
                if pend_drain is not None and si in (3, 5, 7):
                    emit_drain(*pend_drain.pop(0))
                    if not pend_drain:
                        pend_drain = None
                if si >= LAG:
                    emit_pv(pend.pop(0), xps_a, xps_b, xps_c)
                if qh >= 1 and si in (10, 16, 22, 28):
                    base = {10: 0, 16: 2, 22: 4, 28: 6}[si]
                    emit_out_piece(qh - 1, base)
                    emit_out_piece(qh - 1, base + 1)
            for state in pend:
                emit_pv(state, xps_a, xps_b, xps_c)
            pend_drain = [(xps_a[:], xTa[qh][0:64, :]),
                          (xps_b[:], xTa[qh][64:128, :]),
                          (xps_c[:], xTb[qh][0:64, :])]
        for xps, dst in pend_drain:
            emit_drain(xps, dst)
        for piece in range(8):
            emit_out_piece(NQH - 1, piece)

    nc.compile()
    return nc


_NC_CACHE = {}


def _get_nc():
    if "nc" not in _NC_CACHE:
        _NC_CACHE["nc"] = build_nc()
    return _NC_CACHE["nc"]


def make_in_maps(query, key, value, Wq, Wk, bk, Wv, Wp):
    in_maps = []
    for i in range(N_CORES):
        b, g = i // 4, i % 4
        gs = slice(GD * g, GD * (g + 1))
        in_maps.append({
            "query": query[b], "key": key[b], "value": value[b],
            "Wq": Wq[gs, :], "Wk": Wk[gs, :], "bk": bk[gs],
            "Wv": Wv[gs, :], "Wp": Wp[:, gs],
        })
    return [{k: np.ascontiguousarray(v, dtype=np.float32)
             for k, v in m.items()} for m in in_maps]


def reduce_out(res, bp):
    out = np.empty((B, N, DIM), dtype=np.float32)
    for b in range(B):
        acc = res.results[4 * b]["out"].astype(np.float32).copy()
        for g in range(1, 4):
            acc += res.results[4 * b + g]["out"]
        out[b] = acc + bp
    return out


def kernel(query, key, value, Wq, Wk, bk, Wv, Wp, bp, _results_hook=None):
    args = [np.asarray(a, dtype=np.float32)
            for a in (query, key, value, Wq, Wk, bk, Wv, Wp)]
    nc = _get_nc()
    in_maps = make_in_maps(*args)
    res = bass_utils.run_bass_kernel_spmd(nc, in_maps,
                                          core_ids=list(range(N_CORES)))
    if _results_hook is not None:
        _results_hook(res)
    return reduce_out(res, np.asarray(bp, dtype=np.float32))


# revision 26
# speedup vs baseline: 1.0401x; 1.0070x over previous
"""ActivateAttention Trainium2 kernel — 8 NeuronCores, SPMD, head-sharded.

Sharding: core i handles batch b=i//4 and head-group g=i%4 (3 of the 12
heads: ha=3g, hb=3g+1, hc=3g+2), all 4096 queries, full K/V for its
batch. Each core returns a PARTIAL output x_g @ Wp[:, 192g:192g+192].T;
the host sums the 4 group partials per batch and adds bp.

Per-core pipeline (fp16 compute, f32 PSUM accumulate):
  lead-in: weights + all of V and K projected before attention (x^T via
           PE transposes, W^T.T @ x^T; k gets +bias then exact GELU on
           ACT, grouped so the exp table loads once for attention).
           Input DMAs split across the sync and gpsimd hardware queues;
           transposes rotate through a 3-buf PSUM pool that closes
           before attention and hands its banks to the PV accumulators.
  attn:    8 passes over q-halves (512 cols), merged stream of pair and
           solo steps. Per step the two heads' S^T [128,512] land in
           adjacent PSUM banks from matmuls issued back-to-back into
           DIFFERENT PE row groups (partitions 0-63 / 64-127) so they
           stream concurrently. exp(SCALE*S): ACT exact into pt_a, DVE
           one-instruction Schraudolph exp2 into pt_b (affine f32 ->
           int16 convert, bitcast as fp16) — separate tiles/banks so the
           two engines run in parallel. PV matmuls are emitted one step
           LATE so the in-order PE queue never blocks on the exp
           semaphores. PV accumulates [65,512] per head (ones column ->
           softmax denominators). The solo head hc is row-paired across
           adjacent k-tiles via duplicated qT/kT copies in both halves.
  tail:    per finished q-half: x * recip(denominator) -> xT fp16;
           out_partial = xT.T @ Wp_slice^T (host adds bp).
"""

import numpy as np
from contextlib import ExitStack

from concourse import bass, bacc, mybir, masks, tile
from concourse import bass_utils

F32 = mybir.dt.float32
FP16 = mybir.dt.float16
I16 = mybir.dt.int16
AF = mybir.ActivationFunctionType
ALU = mybir.AluOpType

B = 2
N = 4096
DIM = 768
H = 12
D = 64
SCALE = D ** -0.5            # 1/8
N_CORES = 8
HG = 3                       # heads per core
GD = HG * D                  # 192 output dims per core

NT_K = N // 128              # 32 key row-tiles
NCT = DIM // 128             # 6 input-channel tiles
NQH = N // 512               # 8 query halves
LOG2E = 1.4426950408889634
SCH_A = SCALE * LOG2E * 1024.0
SCH_B = 15.0 * 1024.0 - 46.0


def build_nc() -> bass.Bass:
    nc = bacc.Bacc("TRN2", target_bir_lowering=False, debug=False)

    query = nc.declare_dram_parameter("query", [N, DIM], F32, False).ap()
    key = nc.declare_dram_parameter("key", [N, DIM], F32, False).ap()
    value = nc.declare_dram_parameter("value", [N, DIM], F32, False).ap()
    Wq = nc.declare_dram_parameter("Wq", [GD, DIM], F32, False).ap()
    Wk = nc.declare_dram_parameter("Wk", [GD, DIM], F32, False).ap()
    bk = nc.declare_dram_parameter("bk", [GD], F32, False).ap()
    Wv = nc.declare_dram_parameter("Wv", [GD, DIM], F32, False).ap()
    Wp = nc.declare_dram_parameter("Wp", [DIM, GD], F32, False).ap()
    out = nc.declare_dram_parameter("out", [N, DIM], FP16, True).ap()

    with tile.TileContext(nc) as tc, ExitStack() as ctx:
        # ---------------- persistent SBUF ----------------
        cpool = ctx.enter_context(tc.tile_pool(name="const", bufs=1))
        ident = cpool.tile([128, 128], FP16)
        masks.make_identity(nc, ident[:])
        ones16 = cpool.tile([1, D], FP16)
        nc.vector.memset(ones16[:], 1.0)

        bk_pair = cpool.tile([128, 1], F32)
        nc.sync.dma_start(out=bk_pair[:], in_=bk[0:128].rearrange("(p a) -> p a", a=1))
        bk_solo = cpool.tile([128, 1], F32)
        nc.sync.dma_start(out=bk_solo[0:64, :], in_=bk[128:192].rearrange("(p a) -> p a", a=1))
        nc.sync.dma_start(out=bk_solo[64:128, :], in_=bk[128:192].rearrange("(p a) -> p a", a=1))

        # wqk_t blocks: [0:128) Wq^T pair, [128:256) Wq^T solo dup,
        #               [256:384) Wk^T pair, [384:512) Wk^T solo dup
        wqk_t = cpool.tile([128, NCT, 512], FP16)
        wv_t = cpool.tile([128, NCT, GD], FP16)
        wpA = cpool.tile([128, DIM], FP16)
        wpB = cpool.tile([64, DIM], FP16)

        qT_P = [cpool.tile([128, 1024], FP16, name=f"qTP{j}", tag=f"qTP{j}")
                for j in range(4)]
        qT_S = [cpool.tile([128, 1024], FP16, name=f"qTS{j}", tag=f"qTS{j}")
                for j in range(4)]
        kT_P = [cpool.tile([128, 1024], FP16, name=f"kTP{j}", tag=f"kTP{j}")
                for j in range(4)]
        kT_S = [cpool.tile([128, 1024], FP16, name=f"kTS{j}", tag=f"kTS{j}")
                for j in range(4)]
        v_aug = [cpool.tile([128, HG * 65], FP16, name=f"va{t}", tag=f"va{t}")
                 for t in range(NT_K)]
        xTa = [cpool.tile([128, 512], FP16, name=f"xTa{q}", tag=f"xTa{q}")
               for q in range(NQH)]
        xTb = [cpool.tile([64, 512], FP16, name=f"xTb{q}", tag=f"xTb{q}")
               for q in range(NQH)]

        # ---------------- pools ----------------
        # PSUM banks: spool 2x[128,1024]f32 = 4, rpool 1 canonical = 1,
        # ltp 3 canonical = 3 (lead-in; closed -> apool 3x[65,512]f32 = 3)
        spool = ctx.enter_context(tc.tile_pool(name="spool", bufs=5, space="PSUM"))
        ltp_cm = tc.tile_pool(name="ltp", bufs=3, space="PSUM")
        ltp = ltp_cm.__enter__()
        ldpool = ctx.enter_context(tc.tile_pool(name="ldpool", bufs=6))
        cast_pool = ctx.enter_context(tc.tile_pool(name="cast", bufs=5))
        xt_pool = ctx.enter_context(tc.tile_pool(name="xt", bufs=3))
        pt_pool = ctx.enter_context(tc.tile_pool(name="pt", bufs=9))
        dpool = ctx.enter_context(tc.tile_pool(name="drain", bufs=1))
        opool = ctx.enter_context(tc.tile_pool(name="out", bufs=2))

        # canonical PSUM scratch tile: [128, 512] f32 (exactly one bank),
        # viewed as [128, 6, 128] fp16 for transposes or sliced for misc.
        def canon(pool, name):
            return pool.tile([128, 512], F32, tag="cn", name=name)

        def as_tp(t):
            return t[:].bitcast(FP16)[:, 0:768].rearrange(
                "p (c x) -> p c x", x=128)

        # ---------------- weight prep ----------------
        def load_cast(src_ap, rows, cols, dma_eng):
            wf = ldpool.tile([rows, cols], F32, tag="wf")
            dma_eng.dma_start(out=wf[:], in_=src_ap)
            wb = cast_pool.tile([rows, cols], FP16, tag="wb")
            nc.vector.tensor_copy(wb[:], wf[:])
            return wb

        def transp_to(wb, rows, dst_slices):
            for c in range(NCT):
                tpc = canon(ltp, "tpw")
                tp = tpc[:].bitcast(FP16)[:, 0:rows]
                nc.tensor.transpose(tp, wb[:, 128 * c:128 * (c + 1)],
                                    ident[:rows, :rows])
                for dst in dst_slices:
                    nc.vector.tensor_copy(dst(c), tp)

        for wsrc, col0 in ((Wq, 0), (Wk, 256)):
            wb = load_cast(wsrc[0:128, :], 128, DIM, nc.sync)
            transp_to(wb, 128, [lambda c, col0=col0: wqk_t[:, c, col0:col0 + 128]])
            wb = load_cast(wsrc[128:192, :], 64, DIM, nc.sync)
            transp_to(wb, 64, [
                lambda c, col0=col0: wqk_t[:, c, col0 + 128:col0 + 192],
                lambda c, col0=col0: wqk_t[:, c, col0 + 192:col0 + 256]])
        wb = load_cast(Wv[0:128, :], 128, DIM, nc.gpsimd)
        transp_to(wb, 128, [lambda c: wv_t[:, c, 0:128]])
        wb = load_cast(Wv[128:192, :], 64, DIM, nc.gpsimd)
        transp_to(wb, 64, [lambda c: wv_t[:, c, 128:192]])
        for r in range(NCT):
            wb = load_cast(Wp[128 * r:128 * (r + 1), :], 128, GD, nc.gpsimd)
            tpc = canon(ltp, "tpp")
            tp = tpc[:].bitcast(FP16)[:, 0:128]
            nc.tensor.transpose(tp, wb[:, 0:128], ident[:])
            nc.vector.tensor_copy(wpA[:, 128 * r:128 * (r + 1)], tp)
            tpc2 = canon(ltp, "tpp2")
            tp2 = tpc2[:].bitcast(FP16)[0:64, 128:256]
            nc.tensor.transpose(tp2, wb[:, 128:192], ident[:])
            nc.vector.tensor_copy(wpB[:, 128 * r:128 * (r + 1)], tp2)

        # ---------------- input row-tile -> x^T ----------------
        def emit_x_rowtile(src_ap, xt, j, t, dma_eng, cp_eng, cast_eng, tp_pool):
            row0 = 1024 * j + 128 * t
            xf = ldpool.tile([128, DIM], F32, tag="xf")
            dma_eng.dma_start(out=xf[:], in_=src_ap[row0:row0 + 128, :])
            xb = cast_pool.tile([128, DIM], FP16, tag="xb")
            cast_eng(xb[:], xf[:])
            tpc = canon(tp_pool, "tpx")
            tp = as_tp(tpc)
            for c in range(NCT):
                nc.tensor.transpose(tp[:, c, :], xb[:, 128 * c:128 * (c + 1)],
                                    ident[:])
            cp_eng(xt[:, :, 128 * t:128 * (t + 1)], tp)

        def emit_proj_block(xt, wcol0, dst, gelu, bias, cp_eng=None):
            for h2 in range(2):
                pp = canon(spool, f"pp{h2}")
                for c in range(NCT):
                    nc.tensor.matmul(
                        pp[:], wqk_t[:, c, wcol0:wcol0 + 128],
                        xt[:, c, 512 * h2:512 * (h2 + 1)],
                        start=(c == 0), stop=(c == NCT - 1))
                dsl = dst[:, 512 * h2:512 * (h2 + 1)]
                if gelu:
                    nc.scalar.activation(dsl, pp[:], AF.Gelu, bias=bias,
                                         scale=1.0)
                else:
                    (cp_eng or nc.vector.tensor_copy)(dsl, pp[:])

        def emit_v_tile(t):
            vf = ldpool.tile([128, DIM], F32, tag="vf")
            nc.gpsimd.dma_start(out=vf[:], in_=value[128 * t:128 * (t + 1), :])
            vb = cast_pool.tile([128, DIM], FP16, tag="vb")
            nc.vector.tensor_copy(vb[:], vf[:])
            vt = cast_pool.tile([128, NCT, 128], FP16, tag="vt")
            tpc = canon(spool, "tpv")
            tpv = as_tp(tpc)
            for c in range(NCT):
                nc.tensor.transpose(tpv[:, c, :], vb[:, 128 * c:128 * (c + 1)],
                                    ident[:])
            nc.vector.tensor_copy(vt[:], tpv)
            pvc = canon(spool, "pvt")
            pv = pvc[:, 0:GD]
            for c in range(NCT):
                nc.tensor.matmul(pv, vt[:, c, :], wv_t[:, c, :],
                                 start=(c == 0), stop=(c == NCT - 1))
            dst3 = v_aug[t][:].rearrange("p (h w) -> p h w", w=65)
            nc.scalar.copy(dst3[:, :, 0:64],
                           pv.rearrange("p (h w) -> p h w", w=64))
            nc.vector.memset(dst3[:, :, 64:65], 1.0)

        # ---------------- attention ----------------
        def emit_qk_exp(qh, st, dve_only=False):
            kind, idx = st
            q0 = 512 * (qh % 2)
            jq = qh // 2
            if kind == "P":
                c0 = 128 * (idx % 8)
                jk = idx // 8
                la = kT_P[jk][0:64, c0:c0 + 128]
                lb = kT_P[jk][64:128, c0:c0 + 128]
                ra = qT_P[jq][0:64, q0:q0 + 512]
                rb = qT_P[jq][64:128, q0:q0 + 512]
            else:
                kta, ktb = 2 * idx, 2 * idx + 1
                la = kT_S[kta // 8][0:64, 128 * (kta % 8):128 * (kta % 8) + 128]
                lb = kT_S[ktb // 8][64:128, 128 * (ktb % 8):128 * (ktb % 8) + 128]
                ra = qT_S[jq][0:64, q0:q0 + 512]
                rb = qT_S[jq][64:128, q0:q0 + 512]
            slot_a = canon(spool, f"sa{kind}{qh}_{idx}")
            slot_b = canon(spool, f"sb{kind}{qh}_{idx}")
            nc.tensor.matmul(slot_a[:], la, ra, start=True, stop=True)
            nc.tensor.matmul(slot_b[:], lb, rb, start=True, stop=True)
            pta = pt_pool.tile([128, 512], FP16, tag="pt", name=f"pa{kind}{qh}_{idx}")
            ptb = pt_pool.tile([128, 512], FP16, tag="pt", name=f"pb{kind}{qh}_{idx}")
            if dve_only:
                nc.vector.tensor_scalar(pta[:].bitcast(I16), slot_a[:],
                                        SCH_A, SCH_B, ALU.mult, ALU.add)
            else:
                nc.scalar.activation(pta[:], slot_a[:], AF.Exp, scale=SCALE)
            nc.vector.tensor_scalar(ptb[:].bitcast(I16), slot_b[:],
                                    SCH_A, SCH_B, ALU.mult, ALU.add)
            return (kind, idx, pta, ptb)

        def emit_pv(state, xps_a, xps_b, xps_c):
            kind, idx, pta, ptb = state
            if kind == "P":
                va = v_aug[idx][:]
                nc.tensor.matmul(xps_a[:], va[:, 0:65], pta[:],
                                 start=(idx == 0), stop=(idx == NT_K - 1),
                                 skip_group_check=True)
                nc.tensor.matmul(xps_b[:], va[:, 65:130], ptb[:],
                                 start=(idx == 0), stop=(idx == NT_K - 1),
                                 skip_group_check=True)
            else:
                kta, ktb = 2 * idx, 2 * idx + 1
                nc.tensor.matmul(xps_c[:], v_aug[kta][:, 130:195], pta[:],
                                 start=(idx == 0), stop=False,
                                 skip_group_check=True)
                nc.tensor.matmul(xps_c[:], v_aug[ktb][:, 130:195], ptb[:],
                                 start=False, stop=(idx == NT_K // 2 - 1),
                                 skip_group_check=True)

        def emit_drain(xps, dst):
            d16 = dpool.tile([1, 512], FP16, tag="d16")
            nc.scalar.copy(d16[:], xps[64:65, :])
            rpc = canon(spool, "rps")
            Rp = rpc[0:D, :]
            nc.tensor.matmul(Rp, ones16[:], d16[:], start=True, stop=True)
            Rs = dpool.tile([D, 512], F32, tag="Rs")
            nc.vector.reciprocal_approx_fast(Rs[:], Rp)
            nc.vector.tensor_tensor(dst, xps[0:64, :], Rs[:], op=ALU.mult)

        _ot_state = {}

        def emit_out_piece(qh, piece):
            tq, hi = piece // 2, piece % 2
            if hi == 0:
                _ot_state["ot"] = opool.tile([128, DIM], FP16, tag="ot",
                                             name=f"ot{qh}_{tq}")
            ot = _ot_state["ot"]
            o0, w = (0, 512) if hi == 0 else (512, 256)
            poc = canon(spool, f"po{qh}_{tq}_{o0}")
            po = poc[:, 0:w]
            nc.tensor.matmul(po, xTa[qh][:, 128 * tq:128 * (tq + 1)],
                             wpA[:, o0:o0 + w], start=True, stop=False)
            nc.tensor.matmul(po, xTb[qh][:, 128 * tq:128 * (tq + 1)],
                             wpB[:, o0:o0 + w], start=False, stop=True)
            if tq % 2 == 0:
                nc.scalar.copy(ot[:, o0:o0 + w], po)
            else:
                nc.vector.tensor_copy(ot[:, o0:o0 + w], po)
            if hi == 1:
                nc.gpsimd.dma_start(
                    out=out[512 * qh + 128 * tq:512 * qh + 128 * (tq + 1), :],
                    in_=ot[:])

        # ---------------- lead-in (pass 0 interleaved, all-DVE exp) -------
        steps = []
        for kt in range(NT_K):
            steps.append(("P", kt))
            if kt % 2 == 1:
                steps.append(("S", kt // 2))
        LAG = 4

        p0_acc = {}
        p0_pend = []

        def p0_accums():
            if not p0_acc:
                for nm in ("a", "b", "c"):
                    cnt = canon(ltp, f"x0{nm}")
                    p0_acc[nm] = cnt[0:65, :]
            return p0_acc["a"], p0_acc["b"], p0_acc["c"]

        kxt = qxt = None
        p0_ready = []

        def p0_run(n):
            for _ in range(min(n, len(p0_ready))):
                p0_pend.append(emit_qk_exp(0, p0_ready.pop(0), dve_only=True))
                if len(p0_pend) > LAG:
                    emit_pv(p0_pend.pop(0), *p0_accums())

        for t in range(NT_K):
            j, r = t // 8, t % 8
            if r == 0:
                kxt = xt_pool.tile([128, NCT, 1024], FP16, tag="xt",
                                   name=f"kxt{j}")
                qxt = xt_pool.tile([128, NCT, 1024], FP16, tag="xt",
                                   name=f"qxt{j}")
            emit_v_tile(t)
            emit_x_rowtile(key, kxt, j, r, nc.sync, nc.scalar.copy,
                           nc.vector.tensor_copy, spool)
            p0_run(1)
            emit_x_rowtile(query, qxt, j, r, nc.sync, nc.scalar.copy,
                           nc.vector.tensor_copy, spool)
            if r % 2 == 1:
                p0_run(1)
            if r == 7:
                emit_proj_block(kxt, 256, kT_P[j], True, bk_pair[:, 0:1])
                emit_proj_block(kxt, 384, kT_S[j], True, bk_solo[:, 0:1])
                emit_proj_block(qxt, 0, qT_P[j], False, None)
                emit_proj_block(qxt, 128, qT_S[j], False, None)
                p0_ready.extend(steps[12 * j:12 * (j + 1)])
        while p0_ready:
            p0_run(4)
        for state in p0_pend:
            emit_pv(state, *p0_accums())
        xa0, xb0, xc0 = p0_accums()
        emit_drain(xa0, xTa[0][0:64, :])
        emit_drain(xb0, xTa[0][64:128, :])
        emit_drain(xc0, xTb[0][0:64, :])

        ltp_cm.__exit__(None, None, None)
        apool = ctx.enter_context(tc.tile_pool(name="apool", bufs=3, space="PSUM"))

        pend_drain = None
        for qh in range(1, NQH):
            xps_a = apool.tile([65, 512], F32, tag="ap", name=f"xa{qh}")
            xps_b = apool.tile([65, 512], F32, tag="ap", name=f"xb{qh}")
            xps_c = apool.tile([65, 512], F32, tag="ap", name=f"xc{qh}")
            pend = []
            for si, st in enumerate(steps):
                pend.append(emit_qk_exp(qh, st))
                if pend_drain is not None and si in (3, 5, 7):
                    emit_drain(*pend_drain.pop(0))
                    if not pend_drain:
                        pend_drain = None
                if si >= LAG:
                    emit_pv(pend.pop(0), xps_a, xps_b, xps_c)
                if qh >= 1 and si in (10, 16, 22, 28):
                    base = {10: 0, 16: 2, 22: 4, 28: 6}[si]
                    emit_out_piece(qh - 1, base)
                    emit_out_piece(qh - 1, base + 1)
            for state in pend:
                emit_pv(state, xps_a, xps_b, xps_c)
            pend_drain = [(xps_a[:], xTa[qh][0:64, :]),
                          (xps_b[:], xTa[qh][64:128, :]),
                          (xps_c[:], xTb[qh][0:64, :])]
        for xps, dst in pend_drain:
            emit_drain(xps, dst)
        for piece in range(8):
            emit_out_piece(NQH - 1, piece)

    nc.compile()
    return nc


_NC_CACHE = {}


def _get_nc():
    if "nc" not in _NC_CACHE:
        _NC_CACHE["nc"] = build_nc()
    return _NC_CACHE["nc"]


def make_in_maps(query, key, value, Wq, Wk, bk, Wv, Wp):
    in_maps = []
    for i in range(N_CORES):
        b, g = i // 4, i % 4
        gs = slice(GD * g, GD * (g + 1))
        in_maps.append({
            "query": query[b], "key": key[b], "value": value[b],
            "Wq": Wq[gs, :], "Wk": Wk[gs, :], "bk": bk[gs],
            "Wv": Wv[gs, :], "Wp": Wp[:, gs],
        })
    return [{k: np.ascontiguousarray(v, dtype=np.float32)
             for k, v in m.items()} for m in in_maps]


def reduce_out(res, bp):
    out = np.empty((B, N, DIM), dtype=np.float32)
    for b in range(B):
        acc = res.results[4 * b]["out"].astype(np.float32).copy()
        for g in range(1, 4):
            acc += res.results[4 * b + g]["out"]
        out[b] = acc + bp
    return out


def kernel(query, key, value, Wq, Wk, bk, Wv, Wp, bp, _results_hook=None):
    args = [np.asarray(a, dtype=np.float32)
            for a in (query, key, value, Wq, Wk, bk, Wv, Wp)]
    nc = _get_nc()
    in_maps = make_in_maps(*args)
    res = bass_utils.run_bass_kernel_spmd(nc, in_maps,
                                          core_ids=list(range(N_CORES)))
    if _results_hook is not None:
        _results_hook(res)
    return reduce_out(res, np.asarray(bp, dtype=np.float32))


# revision 29
# speedup vs baseline: 1.0445x; 1.0043x over previous
"""ActivateAttention Trainium2 kernel — 8 NeuronCores, SPMD, head-sharded.

Sharding: core i handles batch b=i//4 and head-group g=i%4 (3 of the 12
heads: ha=3g, hb=3g+1, hc=3g+2), all 4096 queries, full K/V for its
batch. Each core returns a PARTIAL output x_g @ Wp[:, 192g:192g+192].T;
the host sums the 4 group partials per batch and adds bp.

Per-core pipeline (fp16 compute, f32 PSUM accumulate):
  lead-in: weights + all of V and K projected before attention (x^T via
           PE transposes, W^T.T @ x^T; k gets +bias then exact GELU on
           ACT, grouped so the exp table loads once for attention).
           Input DMAs split across the sync and gpsimd hardware queues;
           transposes rotate through a 3-buf PSUM pool that closes
           before attention and hands its banks to the PV accumulators.
  attn:    8 passes over q-halves (512 cols), merged stream of pair and
           solo steps. Per step the two heads' S^T [128,512] land in
           adjacent PSUM banks from matmuls issued back-to-back into
           DIFFERENT PE row groups (partitions 0-63 / 64-127) so they
           stream concurrently. exp(SCALE*S): ACT exact into pt_a, DVE
           one-instruction Schraudolph exp2 into pt_b (affine f32 ->
           int16 convert, bitcast as fp16) — separate tiles/banks so the
           two engines run in parallel. PV matmuls are emitted one step
           LATE so the in-order PE queue never blocks on the exp
           semaphores. PV accumulates [65,512] per head (ones column ->
           softmax denominators). The solo head hc is row-paired across
           adjacent k-tiles via duplicated qT/kT copies in both halves.
  tail:    per finished q-half: x * recip(denominator) -> xT fp16;
           out_partial = xT.T @ Wp_slice^T (host adds bp).
"""

import numpy as np
from contextlib import ExitStack

from concourse import bass, bacc, mybir, masks, tile
from concourse import bass_utils

F32 = mybir.dt.float32
FP16 = mybir.dt.float16
I16 = mybir.dt.int16
AF = mybir.ActivationFunctionType
ALU = mybir.AluOpType

B = 2
N = 4096
DIM = 768
H = 12
D = 64
SCALE = D ** -0.5            # 1/8
N_CORES = 8
HG = 3                       # heads per core
GD = HG * D                  # 192 output dims per core

NT_K = N // 128              # 32 key row-tiles
NCT = DIM // 128             # 6 input-channel tiles
NQH = N // 512               # 8 query halves
LOG2E = 1.4426950408889634
SCH_A = SCALE * LOG2E * 1024.0
SCH_B = 15.0 * 1024.0 - 46.0


def build_nc() -> bass.Bass:
    nc = bacc.Bacc("TRN2", target_bir_lowering=False, debug=False)

    query = nc.declare_dram_parameter("query", [N, DIM], F32, False).ap()
    key = nc.declare_dram_parameter("key", [N, DIM], F32, False).ap()
    value = nc.declare_dram_parameter("value", [N, DIM], F32, False).ap()
    Wq = nc.declare_dram_parameter("Wq", [GD, DIM], F32, False).ap()
    Wk = nc.declare_dram_parameter("Wk", [GD, DIM], F32, False).ap()
    bk = nc.declare_dram_parameter("bk", [GD], F32, False).ap()
    Wv = nc.declare_dram_parameter("Wv", [GD, DIM], F32, False).ap()
    Wp = nc.declare_dram_parameter("Wp", [DIM, GD], F32, False).ap()
    out = nc.declare_dram_parameter("out", [N, DIM], FP16, True).ap()

    with tile.TileContext(nc) as tc, ExitStack() as ctx:
        # ---------------- persistent SBUF ----------------
        cpool = ctx.enter_context(tc.tile_pool(name="const", bufs=1))
        ident = cpool.tile([128, 128], FP16)
        masks.make_identity(nc, ident[:])
        ones16 = cpool.tile([1, D], FP16)
        nc.vector.memset(ones16[:], 1.0)

        bk_pair = cpool.tile([128, 1], F32)
        nc.sync.dma_start(out=bk_pair[:], in_=bk[0:128].rearrange("(p a) -> p a", a=1))
        bk_solo = cpool.tile([128, 1], F32)
        nc.sync.dma_start(out=bk_solo[0:64, :], in_=bk[128:192].rearrange("(p a) -> p a", a=1))
        nc.sync.dma_start(out=bk_solo[64:128, :], in_=bk[128:192].rearrange("(p a) -> p a", a=1))

        # wqk_t blocks: [0:128) Wq^T pair, [128:256) Wq^T solo dup,
        #               [256:384) Wk^T pair, [384:512) Wk^T solo dup
        wqk_t = cpool.tile([128, NCT, 512], FP16)
        wv_t = cpool.tile([128, NCT, GD], FP16)
        wpA = cpool.tile([128, DIM], FP16)
        wpB = cpool.tile([64, DIM], FP16)

        qT_P = [cpool.tile([128, 1024], FP16, name=f"qTP{j}", tag=f"qTP{j}")
                for j in range(4)]
        qT_S = [cpool.tile([128, 1024], FP16, name=f"qTS{j}", tag=f"qTS{j}")
                for j in range(4)]
        kT_P = [cpool.tile([128, 1024], FP16, name=f"kTP{j}", tag=f"kTP{j}")
                for j in range(4)]
        kT_S = [cpool.tile([128, 1024], FP16, name=f"kTS{j}", tag=f"kTS{j}")
                for j in range(4)]
        v_aug = [cpool.tile([128, HG * 65], FP16, name=f"va{t}", tag=f"va{t}")
                 for t in range(NT_K)]
        xTa = [cpool.tile([128, 512], FP16, name=f"xTa{q}", tag=f"xTa{q}")
               for q in range(NQH)]
        xTb = [cpool.tile([64, 512], FP16, name=f"xTb{q}", tag=f"xTb{q}")
               for q in range(NQH)]

        # ---------------- pools ----------------
        # PSUM banks: spool 2x[128,1024]f32 = 4, rpool 1 canonical = 1,
        # ltp 3 canonical = 3 (lead-in; closed -> apool 3x[65,512]f32 = 3)
        spool = ctx.enter_context(tc.tile_pool(name="spool", bufs=5, space="PSUM"))
        ltp_cm = tc.tile_pool(name="ltp", bufs=3, space="PSUM")
        ltp = ltp_cm.__enter__()
        ldpool = ctx.enter_context(tc.tile_pool(name="ldpool", bufs=6))
        cast_pool = ctx.enter_context(tc.tile_pool(name="cast", bufs=5))
        xt_pool = ctx.enter_context(tc.tile_pool(name="xt", bufs=3))
        pt_pool = ctx.enter_context(tc.tile_pool(name="pt", bufs=9))
        dpool = ctx.enter_context(tc.tile_pool(name="drain", bufs=1))
        opool = ctx.enter_context(tc.tile_pool(name="out", bufs=2))

        # canonical PSUM scratch tile: [128, 512] f32 (exactly one bank),
        # viewed as [128, 6, 128] fp16 for transposes or sliced for misc.
        def canon(pool, name):
            return pool.tile([128, 512], F32, tag="cn", name=name)

        def as_tp(t):
            return t[:].bitcast(FP16)[:, 0:768].rearrange(
                "p (c x) -> p c x", x=128)

        # ---------------- weight prep ----------------
        def load_cast(src_ap, rows, cols, dma_eng):
            wf = ldpool.tile([rows, cols], F32, tag="wf")
            dma_eng.dma_start(out=wf[:], in_=src_ap)
            wb = cast_pool.tile([rows, cols], FP16, tag="wb")
            nc.vector.tensor_copy(wb[:], wf[:])
            return wb

        def transp_to(wb, rows, dst_slices):
            for c in range(NCT):
                tpc = canon(ltp, "tpw")
                tp = tpc[:].bitcast(FP16)[:, 0:rows]
                nc.tensor.transpose(tp, wb[:, 128 * c:128 * (c + 1)],
                                    ident[:rows, :rows])
                for dst in dst_slices:
                    nc.vector.tensor_copy(dst(c), tp)

        for wsrc, col0 in ((Wq, 0), (Wk, 256)):
            wb = load_cast(wsrc[0:128, :], 128, DIM, nc.sync)
            transp_to(wb, 128, [lambda c, col0=col0: wqk_t[:, c, col0:col0 + 128]])
            wb = load_cast(wsrc[128:192, :], 64, DIM, nc.sync)
            transp_to(wb, 64, [
                lambda c, col0=col0: wqk_t[:, c, col0 + 128:col0 + 192],
                lambda c, col0=col0: wqk_t[:, c, col0 + 192:col0 + 256]])
        wb = load_cast(Wv[0:128, :], 128, DIM, nc.gpsimd)
        transp_to(wb, 128, [lambda c: wv_t[:, c, 0:128]])
        wb = load_cast(Wv[128:192, :], 64, DIM, nc.gpsimd)
        transp_to(wb, 64, [lambda c: wv_t[:, c, 128:192]])
        for r in range(NCT):
            wb = load_cast(Wp[128 * r:128 * (r + 1), :], 128, GD, nc.gpsimd)
            tpc = canon(ltp, "tpp")
            tp = tpc[:].bitcast(FP16)[:, 0:128]
            nc.tensor.transpose(tp, wb[:, 0:128], ident[:])
            nc.vector.tensor_copy(wpA[:, 128 * r:128 * (r + 1)], tp)
            tpc2 = canon(ltp, "tpp2")
            tp2 = tpc2[:].bitcast(FP16)[0:64, 128:256]
            nc.tensor.transpose(tp2, wb[:, 128:192], ident[:])
            nc.vector.tensor_copy(wpB[:, 128 * r:128 * (r + 1)], tp2)

        # ---------------- input row-tile -> x^T ----------------
        def emit_x_rowtile(src_ap, xt, j, t, dma_eng, cp_eng, cast_eng, tp_pool):
            row0 = 1024 * j + 128 * t
            xf = ldpool.tile([128, DIM], F32, tag="xf")
            dma_eng.dma_start(out=xf[:], in_=src_ap[row0:row0 + 128, :])
            xb = cast_pool.tile([128, DIM], FP16, tag="xb")
            cast_eng(xb[:], xf[:])
            tpc = canon(tp_pool, "tpx")
            tp = as_tp(tpc)
            for c in range(NCT):
                nc.tensor.transpose(tp[:, c, :], xb[:, 128 * c:128 * (c + 1)],
                                    ident[:])
            cp_eng(xt[:, :, 128 * t:128 * (t + 1)], tp)

        def emit_proj_block(xt, wcol0, dst, gelu, bias, cp_eng=None):
            for h2 in range(2):
                pp = canon(spool, f"pp{h2}")
                for c in range(NCT):
                    nc.tensor.matmul(
                        pp[:], wqk_t[:, c, wcol0:wcol0 + 128],
                        xt[:, c, 512 * h2:512 * (h2 + 1)],
                        start=(c == 0), stop=(c == NCT - 1))
                dsl = dst[:, 512 * h2:512 * (h2 + 1)]
                if gelu:
                    nc.scalar.activation(dsl, pp[:], AF.Gelu, bias=bias,
                                         scale=1.0)
                else:
                    (cp_eng or nc.vector.tensor_copy)(dsl, pp[:])

        def emit_v_tile(t):
            vf = ldpool.tile([128, DIM], F32, tag="vf")
            nc.gpsimd.dma_start(out=vf[:], in_=value[128 * t:128 * (t + 1), :])
            vb = cast_pool.tile([128, DIM], FP16, tag="vb")
            nc.vector.tensor_copy(vb[:], vf[:])
            vt = cast_pool.tile([128, NCT, 128], FP16, tag="vt")
            tpc = canon(spool, "tpv")
            tpv = as_tp(tpc)
            for c in range(NCT):
                nc.tensor.transpose(tpv[:, c, :], vb[:, 128 * c:128 * (c + 1)],
                                    ident[:])
            nc.vector.tensor_copy(vt[:], tpv)
            pvc = canon(spool, "pvt")
            pv = pvc[:, 0:GD]
            for c in range(NCT):
                nc.tensor.matmul(pv, vt[:, c, :], wv_t[:, c, :],
                                 start=(c == 0), stop=(c == NCT - 1))
            dst3 = v_aug[t][:].rearrange("p (h w) -> p h w", w=65)
            nc.scalar.copy(dst3[:, :, 0:64],
                           pv.rearrange("p (h w) -> p h w", w=64))
            nc.vector.memset(dst3[:, :, 64:65], 1.0)

        # ---------------- attention ----------------
        def emit_qk_exp(qh, st, dve_only=False):
            kind, idx = st
            q0 = 512 * (qh % 2)
            jq = qh // 2
            if kind == "P":
                c0 = 128 * (idx % 8)
                jk = idx // 8
                la = kT_P[jk][0:64, c0:c0 + 128]
                lb = kT_P[jk][64:128, c0:c0 + 128]
                ra = qT_P[jq][0:64, q0:q0 + 512]
                rb = qT_P[jq][64:128, q0:q0 + 512]
            else:
                kta, ktb = 2 * idx, 2 * idx + 1
                la = kT_S[kta // 8][0:64, 128 * (kta % 8):128 * (kta % 8) + 128]
                lb = kT_S[ktb // 8][64:128, 128 * (ktb % 8):128 * (ktb % 8) + 128]
                ra = qT_S[jq][0:64, q0:q0 + 512]
                rb = qT_S[jq][64:128, q0:q0 + 512]
            slot_a = canon(spool, f"sa{kind}{qh}_{idx}")
            slot_b = canon(spool, f"sb{kind}{qh}_{idx}")
            nc.tensor.matmul(slot_a[:], la, ra, start=True, stop=True)
            nc.tensor.matmul(slot_b[:], lb, rb, start=True, stop=True)
            pta = pt_pool.tile([128, 512], FP16, tag="pt", name=f"pa{kind}{qh}_{idx}")
            ptb = pt_pool.tile([128, 512], FP16, tag="pt", name=f"pb{kind}{qh}_{idx}")
            if dve_only:
                nc.vector.tensor_scalar(pta[:].bitcast(I16), slot_a[:],
                                        SCH_A, SCH_B, ALU.mult, ALU.add)
            else:
                nc.scalar.activation(pta[:], slot_a[:], AF.Exp, scale=SCALE)
            nc.vector.tensor_scalar(ptb[:].bitcast(I16), slot_b[:],
                                    SCH_A, SCH_B, ALU.mult, ALU.add)
            return (kind, idx, pta, ptb)

        def emit_pv(state, xps_a, xps_b, xps_c):
            kind, idx, pta, ptb = state
            if kind == "P":
                va = v_aug[idx][:]
                nc.tensor.matmul(xps_a[:], va[:, 0:65], pta[:],
                                 start=(idx == 0), stop=(idx == NT_K - 1),
                                 skip_group_check=True)
                nc.tensor.matmul(xps_b[:], va[:, 65:130], ptb[:],
                                 start=(idx == 0), stop=(idx == NT_K - 1),
                                 skip_group_check=True)
            else:
                kta, ktb = 2 * idx, 2 * idx + 1
                nc.tensor.matmul(xps_c[:], v_aug[kta][:, 130:195], pta[:],
                                 start=(idx == 0), stop=False,
                                 skip_group_check=True)
                nc.tensor.matmul(xps_c[:], v_aug[ktb][:, 130:195], ptb[:],
                                 start=False, stop=(idx == NT_K // 2 - 1),
                                 skip_group_check=True)

        def emit_drain(xps, dst):
            d16 = dpool.tile([1, 512], FP16, tag="d16")
            nc.scalar.copy(d16[:], xps[64:65, :])
            rpc = canon(spool, "rps")
            Rp = rpc[0:D, :]
            nc.tensor.matmul(Rp, ones16[:], d16[:], start=True, stop=True)
            Rs = dpool.tile([D, 512], F32, tag="Rs")
            nc.vector.reciprocal_approx_fast(Rs[:], Rp)
            nc.vector.tensor_tensor(dst, xps[0:64, :], Rs[:], op=ALU.mult)

        _ot_state = {}

        def emit_out_piece(qh, piece):
            tq, hi = piece // 2, piece % 2
            if hi == 0:
                _ot_state["ot"] = opool.tile([128, DIM], FP16, tag="ot",
                                             name=f"ot{qh}_{tq}")
            ot = _ot_state["ot"]
            o0, w = (0, 512) if hi == 0 else (512, 256)
            poc = canon(spool, f"po{qh}_{tq}_{o0}")
            po = poc[:, 0:w]
            nc.tensor.matmul(po, xTa[qh][:, 128 * tq:128 * (tq + 1)],
                             wpA[:, o0:o0 + w], start=True, stop=False)
            nc.tensor.matmul(po, xTb[qh][:, 128 * tq:128 * (tq + 1)],
                             wpB[:, o0:o0 + w], start=False, stop=True)
            if tq % 2 == 0:
                nc.scalar.copy(ot[:, o0:o0 + w], po)
            else:
                nc.vector.tensor_copy(ot[:, o0:o0 + w], po)
            if hi == 1:
                nc.gpsimd.dma_start(
                    out=out[512 * qh + 128 * tq:512 * qh + 128 * (tq + 1), :],
                    in_=ot[:])

        # ---------------- lead-in (pass 0 interleaved, all-DVE exp) -------
        steps = []
        for kt in range(NT_K):
            steps.append(("P", kt))
            if kt % 2 == 1:
                steps.append(("S", kt // 2))
        LAG = 4

        p0_acc = {}
        p0_pend = []

        def p0_accums():
            if not p0_acc:
                for nm in ("a", "b", "c"):
                    cnt = canon(ltp, f"x0{nm}")
                    p0_acc[nm] = cnt[0:65, :]
            return p0_acc["a"], p0_acc["b"], p0_acc["c"]

        kxt = qxt = None
        p0_ready = []

        def p0_run(n):
            for _ in range(min(n, len(p0_ready))):
                p0_pend.append(emit_qk_exp(0, p0_ready.pop(0), dve_only=True))
                if len(p0_pend) > LAG:
                    emit_pv(p0_pend.pop(0), *p0_accums())

        for t in range(NT_K):
            j, r = t // 8, t % 8
            if r == 0:
                kxt = xt_pool.tile([128, NCT, 1024], FP16, tag="xt",
                                   name=f"kxt{j}")
                qxt = xt_pool.tile([128, NCT, 1024], FP16, tag="xt",
                                   name=f"qxt{j}")
            emit_v_tile(t)
            emit_x_rowtile(key, kxt, j, r, nc.sync, nc.scalar.copy,
                           nc.vector.tensor_copy, spool)
            p0_run(1)
            emit_x_rowtile(query, qxt, j, r, nc.sync, nc.scalar.copy,
                           nc.vector.tensor_copy, spool)
            if r % 2 == 1:
                p0_run(1)
            if r == 7:
                emit_proj_block(kxt, 256, kT_P[j], True, bk_pair[:, 0:1])
                emit_proj_block(kxt, 384, kT_S[j], True, bk_solo[:, 0:1])
                emit_proj_block(qxt, 0, qT_P[j], False, None)
                emit_proj_block(qxt, 128, qT_S[j], False, None)
                p0_ready.extend(steps[12 * j:12 * (j + 1)])
        while p0_ready:
            p0_run(4)
        for state in p0_pend:
            emit_pv(state, *p0_accums())
        xa0, xb0, xc0 = p0_accums()
        emit_drain(xa0, xTa[0][0:64, :])
        emit_drain(xb0, xTa[0][64:128, :])
        emit_drain(xc0, xTb[0][0:64, :])

        ltp_cm.__exit__(None, None, None)
        apool = ctx.enter_context(tc.tile_pool(name="apool", bufs=3, space="PSUM"))

        pend_drain = None
        for qh in range(1, NQH):
            xps_a = apool.tile([65, 512], F32, tag="ap", name=f"xa{qh}")
            xps_b = apool.tile([65, 512], F32, tag="ap", name=f"xb{qh}")
            xps_c = apool.tile([65, 512], F32, tag="ap", name=f"xc{qh}")
            pend = []
            for si, st in enumerate(steps):
                pend.append(emit_qk_exp(qh, st))
                if pend_drain is not None and si in (3, 5, 7):
                    emit_drain(*pend_drain.pop(0))
                    if not pend_drain:
                        pend_drain = None
                if si >= LAG:
                    emit_pv(pend.pop(0), xps_a, xps_b, xps_c)
                if qh >= 1 and si in (10, 16, 22, 28):
                    base = {10: 0, 16: 2, 22: 4, 28: 6}[si]
                    emit_out_piece(qh - 1, base)
                    emit_out_piece(qh - 1, base + 1)
            for state in pend:
                emit_pv(state, xps_a, xps_b, xps_c)
            pend_drain = [(xps_a[:], xTa[qh][0:64, :]),
                          (xps_b[:], xTa[qh][64:128, :]),
                          (xps_c[:], xTb[qh][0:64, :])]
        for xps, dst in pend_drain:
            emit_drain(xps, dst)
        for piece in range(8):
            emit_out_piece(NQH - 1, piece)

    nc.compile()
    return nc


_NC_CACHE = {}


def _get_nc():
    if "nc" not in _NC_CACHE:
        _NC_CACHE["nc"] = build_nc()
    return _NC_CACHE["nc"]


def make_in_maps(query, key, value, Wq, Wk, bk, Wv, Wp):
    in_maps = []
    for i in range(N_CORES):
        b, g = i // 4, i % 4
        gs = slice(GD * g, GD * (g + 1))
        in_maps.append({
            "query": query[b], "key": key[b], "value": value[b],
            "Wq": Wq[gs, :], "Wk": Wk[gs, :], "bk": bk[gs],
            "Wv": Wv[gs, :], "Wp": Wp[:, gs],
        })
    return [{k: np.ascontiguousarray(v, dtype=np.float32)
             for k, v in m.items()} for m in in_maps]


def reduce_out(res, bp):
    out = np.empty((B, N, DIM), dtype=np.float32)
    for b in range(B):
        acc = res.results[4 * b]["out"].astype(np.float32).copy()
        for g in range(1, 4):
            acc += res.results[4 * b + g]["out"]
        out[b] = acc + bp
    return out


def kernel(query, key, value, Wq, Wk, bk, Wv, Wp, bp, _results_hook=None):
    args = [np.asarray(a, dtype=np.float32)
            for a in (query, key, value, Wq, Wk, bk, Wv, Wp)]
    nc = _get_nc()
    in_maps = make_in_maps(*args)
    res = bass_utils.run_bass_kernel_spmd(nc, in_maps,
                                          core_ids=list(range(N_CORES)))
    if _results_hook is not None:
        _results_hook(res)
    return reduce_out(res, np.asarray(bp, dtype=np.float32))
